# revision 59
# baseline (speedup 1.0000x reference)
"""Trainium2 Bass kernel for the GNN message-passing module (v2, bf16).

Edges sorted by destination agent, sharded across 8 cores as contiguous
agent ranges.  Per core, sorted edges are cut into tiles of <=512 edges
whose agents form disjoint windows of <=128 consecutive agents covering the
core's agent range.  All edge activations are feature-major [128 x 512]
bf16; GroupNorm means are folded into centered weights; inverse-stddev is
one scalar-engine Rsqrt; the c1-GN scale is computed per-edge-block with
tiny column-sum matmuls and applied as a per-partition scalar while
evacuating the edge-major Wc2 matmul.  Scatter-add is a 0/1 segment matmul
into per-window PSUM, stored window-ordered and gathered back to agent
order in the per-agent tail.
"""

import sys

sys.path.insert(0, "/opt/trn_rl_repo")

import numpy as np
import ml_dtypes
from contextlib import ExitStack

import concourse.bass as bass
import concourse.tile as tile
from concourse import bacc
from concourse import mybir
from concourse.bass import IndirectOffsetOnAxis
from concourse.bass_utils import run_bass_kernel_spmd
from concourse.masks import make_identity

AF = mybir.ActivationFunctionType
ALU = mybir.AluOpType
F32 = mybir.dt.float32
FR = mybir.dt.float32r
BF16 = mybir.dt.bfloat16
I32 = mybir.dt.int32
BF = ml_dtypes.bfloat16

P = 128
TE = 512
EPS = 1e-5
NCORES = 8
N_AGT = 50000
N_CTX = 100000


def _bf(x):
    return np.ascontiguousarray(np.asarray(x, np.float32)).astype(BF)


def _center(lhsT):
    return (lhsT - lhsT.mean(axis=1, keepdims=True)).astype(np.float32)


# ----------------------------------------------------------------------------
# host-side preparation
# ----------------------------------------------------------------------------

def _plan_core(his, a_start, a_end):
    """Cut a core's sorted edge list into tiles: (e0, ne, A0, na) with
    ne<=TE edges, na<=P agents; windows disjoint, covering [a_start,a_end)."""
    tiles = []
    ne_total = len(his)
    if ne_total:
        starts = np.flatnonzero(np.r_[True, his[1:] != his[:-1]])
        ends = np.r_[starts[1:], ne_total]
        agents = his[starts]
    else:
        starts = ends = agents = np.array([], dtype=np.int64)

    cur_e0 = 0
    cur_A0 = a_start
    for g in range(len(starts)):
        a, gs, ge = int(agents[g]), int(starts[g]), int(ends[g])
        assert ge - gs <= TE, f"agent degree {ge - gs} > {TE}"
        if (ge - cur_e0 > TE) or (a - cur_A0 >= P):
            na = min(a - cur_A0, P)
            tiles.append((cur_e0, gs - cur_e0, cur_A0, na))
            cur_e0 = gs
            cur_A0 += na
            while a - cur_A0 >= P:
                tiles.append((cur_e0, 0, cur_A0, P))
                cur_A0 += P
    while True:
        na = min(a_end - cur_A0, P)
        tiles.append((cur_e0, ne_total - cur_e0, cur_A0, na))
        cur_e0 = ne_total
        cur_A0 += na
        if cur_A0 >= a_end:
            break
    return tiles


def _prepare(agts, ctx, agt_ctrs, ctx_ctrs, hi, wi):
    E = hi.shape[0]
    order = np.argsort(hi, kind="stable")
    his_all = hi[order]
    wis_all = wi[order]

    cuts = [0]
    for c in range(1, NCORES):
        p = c * E // NCORES
        while p < E and his_all[p] == his_all[p - 1]:
            p += 1
        cuts.append(p)
    cuts.append(E)

    a_bounds = [0]
    for c in range(1, NCORES):
        p = cuts[c]
        a_bounds.append(int(his_all[p]) if p < E else N_AGT)
    a_bounds.append(N_AGT)

    cores = []
    for c in range(NCORES):
        e0, e1 = cuts[c], cuts[c + 1]
        cores.append(dict(his=his_all[e0:e1], wis=wis_all[e0:e1],
                          a_start=a_bounds[c], a_end=a_bounds[c + 1]))

    plans = [_plan_core(co["his"], co["a_start"], co["a_end"]) for co in cores]
    nT = max(len(p) for p in plans)
    nT = ((nT + 7) // 8) * 8  # multiple of 8 for batched stores/loads
    nA_max = max(co["a_end"] - co["a_start"] for co in cores)
    nAC = (nA_max + TE - 1) // TE
    napad = nAC * TE
    nACB = napad // P

    dd_all = (agt_ctrs[his_all] - ctx_ctrs[wis_all]).astype(np.float32)

    in_maps = []
    for c, (co, plan) in enumerate(zip(cores, plans)):
        his, wis = co["his"], co["wis"]
        a_start = co["a_start"]
        nA = co["a_end"] - a_start
        e_base = cuts[c]
        n_real = len(plan)

        e0s = np.array([t[0] for t in plan], dtype=np.int64)
        nes = np.array([t[1] for t in plan], dtype=np.int64)
        A0s = np.array([t[2] for t in plan], dtype=np.int64)
        nas = np.array([t[3] for t in plan], dtype=np.int64)

        tidx = np.repeat(np.arange(n_real), nes)
        j = np.arange(len(his)) - np.repeat(e0s, nes)
        loc = his - np.repeat(A0s, nes)
        slot = tidx * TE + j

        dd = np.zeros((2, nT * TE), np.float32)
        dd[0, slot] = dd_all[e_base:e_base + len(his), 0]
        dd[1, slot] = dd_all[e_base:e_base + len(his), 1]


        ctxg = np.zeros((P, nT * TE), BF)
        ctxg[:, slot] = ctx[wis].astype(BF).T

        stm = np.zeros((P, nT * TE), BF)  # st: [loc, t*TE + j] = 1
        stm[loc, slot] = 1.0
        ssm = np.zeros((P, nT * TE), BF)  # ss: [j%P, t*TE + (j//P)*P+loc] = 1
        ssm[j % P, tidx * TE + (j // P) * P + loc] = 1.0

        # qwin gather rows into qbt-flat [(a%P)*nACB + a//P]
        jj = np.arange(P)[None, :]
        a_w = np.clip(A0s[:, None] - a_start + jj, 0, napad - 1)  # [n_real,P]
        widx = np.zeros((nT, P), np.int32)
        widx[:n_real] = (a_w % P) * nACB + a_w // P
        widx = np.ascontiguousarray(widx.T)  # [P, nT]

        # stage-C gather rows into partial-flat [loc*nT + t] per local agent
        t_of = np.zeros(napad, np.int64)
        l_of = np.zeros(napad, np.int64)
        for t in range(n_real):
            A0, na = int(A0s[t]), int(nas[t])
            if na > 0:
                aa = np.arange(A0 - a_start, A0 - a_start + na)
                t_of[aa] = t
                l_of[aa] = np.arange(na)
        gflat = l_of * nT + t_of  # [napad]
        gidx = np.ascontiguousarray(
            gflat.reshape(nACB, P).T.astype(np.int32))  # [P, nACB]

        agtsT16 = np.zeros((P, napad), np.float32)
        agtsT16[:, :nA] = agts[a_start:co["a_end"]].T
        agtsT16 = agtsT16.astype(BF)

        in_maps.append(dict(
            dd=_bf(dd), ctxg=ctxg, stm=stm, ssm=ssm,
            widx=widx, gidx=gidx, agtsT16=agtsT16))

    # gate for interleaving stage C into stage B: cgate[ch] = last tile
    # (max over cores) whose agent window intersects chunk ch's agents
    cgate = []
    for ch in range(nAC):
        g = 0
        for co, plan in zip(cores, plans):
            nA = co["a_end"] - co["a_start"]
            last = min((ch + 1) * TE, nA) - 1
            if last < ch * TE:
                continue
            for ti, (_, _, A0, na) in enumerate(plan):
                if A0 - co["a_start"] <= last < A0 - co["a_start"] + na:
                    g = max(g, ti)
                    break
        cgate.append(g)

    meta = dict(nT=nT, nAC=nAC, napad=napad, nACB=nACB, a_bounds=a_bounds,
                cgate=tuple(cgate))
    return in_maps, meta


WB16 = ["wd2c", "wqc", "w1a", "w1b", "w1c", "wc2r", "wlc16", "umat16",
        "wa16c"]


def _prep_weights(Wd1, bd1, Wd2, Wq, Wc1, Wc2, Wa, Wl):
    w = {}
    parts = {
        "wd2c": _center(Wd2.T), "wqc": _center(Wq.T),
        "w1a": _center(Wc1[:, 0:P].T), "w1b": _center(Wc1[:, P:2 * P].T),
        "w1c": _center(Wc1[:, 2 * P:3 * P].T), "wc2r": _center(Wc2.T),
        "wlc16": _center(Wl.T),
        "umat16": np.full((P, P), 1.0 / P, np.float32),
        "wa16c": _center(Wa.T),
    }
    w["wb16"] = _bf(np.concatenate([parts[k] for k in WB16], axis=1))
    w["ident16"] = _bf(np.eye(P, dtype=np.float32))
    w["zero16"] = np.zeros((P, TE), BF)
    w["identf"] = np.eye(P, dtype=np.float32)
    w["wd1aug"] = _bf(Wd1.T)
    w["bd1"] = bd1.astype(np.float32).reshape(P, 1)
    w["onescol"] = _bf(np.full((P, 1), 1.0 / P, np.float32))
    return w


# ----------------------------------------------------------------------------
# device program
# ----------------------------------------------------------------------------

def _build(nT, nAC, napad, fastgn=True, cgate=None):
    if cgate is None:
        cgate = tuple([nT - 1] * nAC)
    nACB = napad // P
    nc = bacc.Bacc(None, target_bir_lowering=False, debug=False)

    tw = {}
    for name, shape, dt in [
        ("wb16", (P, 9 * P), BF16), ("wd1aug", (2, P), BF16),
        ("bd1", (P, 1), F32), ("ident16", (P, P), BF16),
        ("identf", (P, P), FR), ("zero16", (P, TE), BF16),
        ("onescol", (P, 1), BF16),
    ]:
        tw[name] = nc.dram_tensor(name, shape, dt, kind="ExternalInput")
    t_gv = nc.dram_tensor("gv", (P, 10), F32, kind="ExternalInput")

    t_dd = nc.dram_tensor("dd", (2, nT * TE), BF16, kind="ExternalInput")
    t_ctx = nc.dram_tensor("ctxg", (P, nT * TE), BF16, kind="ExternalInput")
    t_stm = nc.dram_tensor("stm", (P, nT * TE), BF16, kind="ExternalInput")
    t_ssm = nc.dram_tensor("ssm", (P, nT * TE), BF16, kind="ExternalInput")
    t_widx = nc.dram_tensor("widx", (P, nT), I32, kind="ExternalInput")
    t_gidx = nc.dram_tensor("gidx", (P, nACB), I32, kind="ExternalInput")
    t_ag16 = nc.dram_tensor("agtsT16", (P, napad), BF16, kind="ExternalInput")

    t_qbt = nc.dram_tensor("qbt", (P, nACB, P), BF16, kind="ExternalOutput")
    t_part = nc.dram_tensor("part", (P, nT, P), BF16, kind="ExternalOutput")
    t_out = nc.dram_tensor("out", (P, napad), F32, kind="ExternalOutput")

    def rsqrt_act(out_ap, in_ap, bias, scale=1.0):
        r = nc.scalar.activation(out_ap, in_ap, AF.Square, bias=bias,
                                 scale=scale)
        r.ins.func = AF.Rsqrt

    with tile.TileContext(nc) as tc, ExitStack() as ctx:
        const = ctx.enter_context(tc.tile_pool(name="const", bufs=1))
        res = ctx.enter_context(tc.tile_pool(name="res", bufs=1))
        io = ctx.enter_context(tc.tile_pool(name="io", bufs=4))
        io_dd = ctx.enter_context(tc.tile_pool(name="io_dd", bufs=2))
        act = ctx.enter_context(tc.tile_pool(name="act", bufs=3))
        act4 = ctx.enter_context(tc.tile_pool(name="act4", bufs=4))
        act5 = ctx.enter_context(tc.tile_pool(name="act5", bufs=5))
        act2 = ctx.enter_context(tc.tile_pool(name="act2", bufs=2))
        ps_mm = ctx.enter_context(
            tc.tile_pool(name="ps_mm", bufs=4 if not fastgn else 3, space="PSUM"))
        ps_vb = ctx.enter_context(
            tc.tile_pool(name="ps_vb", bufs=3 if fastgn else 1, space="PSUM"))
        ps_seg = ctx.enter_context(
            tc.tile_pool(name="ps_seg", bufs=1 if fastgn else 2,
                         space="PSUM"))
        if fastgn:
            ps_ssq = ctx.enter_context(
                tc.tile_pool(name="ps_ssq", bufs=1, space="PSUM"))
        else:
            ps_me = ctx.enter_context(
                tc.tile_pool(name="ps_me", bufs=1, space="PSUM"))

        W = {}
        wb16 = const.tile([P, 9 * P], BF16, tag="wb16")
        nc.sync.dma_start(wb16[:], tw["wb16"][:, :])
        for kk, name in enumerate(WB16):
            W[name] = wb16[:, kk * P:(kk + 1) * P]
        for name, shape, dt in [("wd1aug", (2, P), BF16),
                                ("bd1", (P, 1), F32),
                                ("identf", (P, P), FR),
                                ("onescol", (P, 1), BF16),
                                ("zero16", (P, TE), BF16)]:
            t = const.tile(list(shape), dt, tag=name)
            nc.gpsimd.dma_start(t[:], tw[name][:, :])
            W[name] = t[:]
        gv = const.tile([P, 10], F32, tag="gv")
        nc.sync.dma_start(gv[:], t_gv[:, :])
        gd2w, gd2b = gv[:, 0:1], gv[:, 1:2]
        gqw, gqb = gv[:, 2:3], gv[:, 3:4]
        gc1w, gc1b = gv[:, 4:5], gv[:, 5:6]
        gnw, gnb = gv[:, 6:7], gv[:, 7:8]
        glw, glb = gv[:, 8:9], gv[:, 9:10]


        zero_b = const.tile([P, 1], F32, tag="zero_b")
        nc.gpsimd.memset(zero_b[:], 0.0)
        eps_b = const.tile([P, 1], F32, tag="eps_b")
        nc.gpsimd.memset(eps_b[:], EPS)

        # resident tables (split across queues so startup isn't
        # serialized on SP)
        ag16 = res.tile([P, napad], BF16, tag="ag16")
        h = napad // 2
        nc.sync.dma_start(ag16[:, 0:h], t_ag16[:, 0:h])
        nc.gpsimd.dma_start(ag16[:, h:napad], t_ag16[:, h:napad])
        widx = res.tile([P, nT], I32, tag="widx")
        nc.gpsimd.dma_start(widx[:], t_widx[:, :])
        gidx = res.tile([P, nACB], I32, tag="gidx")
        nc.gpsimd.dma_start(gidx[:], t_gidx[:, :])

        # ---- stage A: per-agent query branch -> qbt table ----
        # modulo-scheduled: ch at i: zq+zqb | i+1: sqq [Pool] | i+2: vbq |
        # i+3: rsq | i+4: q [Pool] | i+5: qbp+qbs | i+6: transposes+evac+
        # store.  zq/qbp freed the step they are produced (ps_mm ring 2).
        sa = {}
        for i in range(nAC + 7):
            if i < nAC:
                ch = i
                sa[ch] = sA = {}
                sl = slice(ch * TE, (ch + 1) * TE)
                zq = ps_mm.tile([P, TE], F32, tag="mm", name=f"zq{ch}")
                nc.tensor.matmul(zq[:], W["wqc"], ag16[:, sl],
                                 start=True, stop=True)
                zqb = act.tile([P, TE], BF16, tag="z2b", name=f"zqb{ch}",
                               bufs=5)
                sA["zqb"] = zqb
                nc.scalar.activation(zqb[:], zq[:], AF.Copy)
            if 0 <= i - 1 < nAC:
                ch = i - 1
                sA = sa[ch]
                sqq = act.tile([P, TE], BF16, tag="sq", name=f"sqq{ch}")
                sA["sqq"] = sqq
                nc.gpsimd.tensor_tensor(sqq[:], sA["zqb"][:], sA["zqb"][:],
                                        op=ALU.mult)
            if 0 <= i - 2 < nAC:
                ch = i - 2
                sA = sa[ch]
                vbq = ps_vb.tile([P, TE], F32, tag="vb", name=f"vbq{ch}")
                sA["vbq"] = vbq
                nc.tensor.matmul(vbq[:], W["umat16"], sA["sqq"][:],
                                 start=True, stop=True)
            if 0 <= i - 3 < nAC:
                ch = i - 3
                sA = sa[ch]
                rsq = act.tile([P, TE], BF16, tag="rs", name=f"rsq{ch}")
                sA["rsq"] = rsq
                rsqrt_act(rsq[:], sA["vbq"][:], eps_b[:])
            if 0 <= i - 4 < nAC:
                ch = i - 4
                sA = sa[ch]
                q = act.tile([P, TE], BF16, tag="hq", name=f"q{ch}")
                sA["q"] = q
                if fastgn:
                    nc.vector.scalar_tensor_tensor(
                        out=q[:], in0=sA["zqb"][:], scalar=0.0,
                        in1=sA["rsq"][:], op0=ALU.max, op1=ALU.mult)
                else:
                    tm = act.tile([P, TE], F32, tag="tmq", name=f"tmq{ch}")
                    nc.vector.tensor_tensor(tm[:], sA["zqb"][:],
                                            sA["rsq"][:], op=ALU.mult)
                    nc.scalar.activation(q[:], tm[:], AF.Relu,
                                         scale=gqw, bias=gqb)
            if 0 <= i - 5 < nAC:
                ch = i - 5
                sA = sa[ch]
                qbp = ps_mm.tile([P, TE], F32, tag="mm", name=f"qbp{ch}")
                nc.tensor.matmul(qbp[:], W["w1b"], sA["q"][:],
                                 start=True, stop=True)
                qbs = act.tile([P, TE], FR, tag="qbs", name=f"qbs{ch}")
                sA["qbs16f"] = qbs
                nc.scalar.activation(qbs[:], qbp[:], AF.Copy)
            if 0 <= i - 6 < nAC:
                ch = i - 6
                sA = sa.pop(ch)
                qam = act.tile([P, 4 * P], BF16, tag="qam", name=f"qam{ch}")
                tpq = ps_vb.tile([P, TE], FR, tag="vb", name=f"qtp{ch}")
                for k in range(4):
                    nc.tensor.transpose(tpq[:, k * P:(k + 1) * P],
                                        sA["qbs16f"][:, k * P:(k + 1) * P],
                                        W["identf"])
                nc.vector.tensor_copy(qam[:], tpq[:])
                nc.sync.dma_start(t_qbt[:, 4 * ch:4 * ch + 4, :], qam[:])

        # ---- stage C: per-agent tail ----
        # modulo-scheduled: ch at i: gather+apz | i+1: transposes+evac+asb |
        # i+3: sqn | i+4: vbn+rsn | i+5: an | i+6: zl+zl16+sql | i+7:
        # vbl+rsl | i+8: tl | i+9: t3+oc+store
        # In the fastgn build the columns are interleaved into the stage-B
        # loop (gated on the part stores each chunk reads); emit_c is also
        # used for the sequential drain after stage B.
        scs = {}

        def emit_c(i):
            # ---- PE ----
            if i < nAC:
                ch = i
                scs[ch] = sC = {}
                sl = slice(ch * TE, (ch + 1) * TE)
                pam = act.tile([P, 4 * P], BF16, tag="pam", name=f"pam{ch}")
                sC["pam"] = pam
                for kk in range(4):
                    nc.gpsimd.indirect_dma_start(
                        out=pam[:, kk * P:(kk + 1) * P], out_offset=None,
                        in_=t_part[:, :, :],
                        in_offset=IndirectOffsetOnAxis(
                            ap=gidx[:, 4 * ch + kk:4 * ch + kk + 1], axis=1))
                apz = ps_mm.tile([P, TE], F32, tag="mm", name=f"apz{ch}")
                sC["apz"] = apz
                nc.tensor.matmul(apz[:], W["wa16c"], ag16[:, sl],
                                 start=True, stop=True)
                pamf = act.tile([P, 4 * P], FR, tag="pamf", name=f"pamf{ch}")
                sC["pamf"] = pamf
                nc.gpsimd.tensor_copy(pamf[:], pam[:])
            if 0 <= i - 1 < nAC:
                ch = i - 1
                sC = scs[ch]
                pfm = act.tile([P, TE], BF16, tag="pfm", name=f"pfm{ch}")
                sC["pfm"] = pfm
                tpp = ps_vb.tile([P, TE], FR, tag="vb", name=f"ptp{ch}")
                for k in range(4):
                    nc.tensor.transpose(tpp[:, k * P:(k + 1) * P],
                                        sC["pamf"][:, k * P:(k + 1) * P],
                                        W["identf"])
                nc.vector.tensor_copy(pfm[:], tpp[:])
                asb = act5.tile([P, TE], BF16, tag="asb", name=f"asb{ch}")
                sC["asb"] = asb
                nc.vector.tensor_tensor(asb[:], sC["apz"][:],
                                        pfm[:], op=ALU.add)
            if 0 <= i - 4 < nAC:
                ch = i - 4
                sC = scs[ch]
                vbn = ps_vb.tile([P, TE], F32, tag="vb", name=f"vbn{ch}")
                nc.tensor.matmul(vbn[:], W["umat16"], sC["sqn"][:],
                                 start=True, stop=True)
                rsn = act.tile([P, TE], BF16, tag="rs", name=f"rsn{ch}")
                sC["rsn"] = rsn
                rsqrt_act(rsn[:], vbn[:], eps_b[:])
            if 0 <= i - 6 < nAC:
                ch = i - 6
                sC = scs[ch]
                zl = ps_vb.tile([P, TE], F32, tag="vb", name=f"zl{ch}")
                sC["zl"] = zl
                nc.tensor.matmul(zl[:], W["wlc16"], sC["an"][:],
                                 start=True, stop=True)
            if 0 <= i - 7 < nAC:
                ch = i - 7
                sC = scs[ch]
                vbl = ps_vb.tile([P, TE], F32, tag="vb", name=f"vbl{ch}")
                nc.tensor.matmul(vbl[:], W["umat16"], sC["sql"][:],
                                 start=True, stop=True)
                rsl = act.tile([P, TE], BF16, tag="rsl", name=f"rsl{ch}")
                sC["rsl"] = rsl
                rsqrt_act(rsl[:], vbl[:], eps_b[:])
            # ---- DVE ----
            if 0 <= i - 3 < nAC:
                ch = i - 3
                sC = scs[ch]
                sqn = act.tile([P, TE], BF16, tag="sq", name=f"sqn{ch}")
                sC["sqn"] = sqn
                nc.gpsimd.tensor_tensor(sqn[:], sC["asb"][:], sC["asb"][:],
                                        op=ALU.mult)

            if 0 <= i - 5 < nAC:
                ch = i - 5
                sC = scs[ch]
                an = act.tile([P, TE], BF16, tag="an", name=f"an{ch}")
                sC["an"] = an
                if fastgn:
                    nc.vector.scalar_tensor_tensor(
                        out=an[:], in0=sC["asb"][:], scalar=0.0,
                        in1=sC["rsn"][:], op0=ALU.max, op1=ALU.mult)
                else:
                    tm = act.tile([P, TE], F32, tag="tmn", name=f"tmn{ch}")
                    nc.vector.tensor_tensor(tm[:], sC["asb"][:], sC["rsn"][:],
                                            op=ALU.mult)
                    nc.scalar.activation(an[:], tm[:], AF.Relu,
                                         scale=gnw, bias=gnb)
            if 0 <= i - 8 < nAC:
                ch = i - 8
                sC = scs[ch]
                tl = act.tile([P, TE], BF16, tag="tl", name=f"tl{ch}")
                sC["tl"] = tl
                nc.gpsimd.tensor_tensor(tl[:], sC["zl16"][:], sC["rsl"][:],
                                        op=ALU.mult)
            # ---- Pool ----
            if 0 <= i - 6 < nAC:
                ch = i - 6
                sC = scs[ch]
                zl16 = act.tile([P, TE], BF16, tag="zl16", name=f"zl16{ch}")
                sC["zl16"] = zl16
                nc.scalar.activation(zl16[:], sC["zl"][:], AF.Copy)
                sql = act.tile([P, TE], BF16, tag="sql", name=f"sql{ch}")
                sC["sql"] = sql
                nc.gpsimd.tensor_tensor(sql[:], zl16[:], zl16[:],
                                        op=ALU.mult)
            if 0 <= i - 9 < nAC:
                ch = i - 9
                sC = scs.pop(ch)
                sl = slice(ch * TE, (ch + 1) * TE)
                tl = sC["tl"]
                if not fastgn:
                    t2 = act.tile([P, TE], F32, tag="t2", name=f"t2{ch}")
                    nc.vector.tensor_scalar(t2[:], tl[:], glw, glb,
                                            op0=ALU.mult, op1=ALU.add)
                    tl = t2
                t3 = act.tile([P, TE], F32, tag="t3", name=f"t3{ch}")
                nc.vector.tensor_tensor(t3[:], tl[:], ag16[:, sl], op=ALU.add)
                oc = act.tile([P, TE], F32, tag="oc", name=f"oc{ch}")
                nc.scalar.activation(oc[:], t3[:], AF.Relu, bias=zero_b[:])
                nc.sync.dma_start(t_out[:, sl], oc[:])

        # ---- stage B: edge tiles ----
        sb = {}
        c_emitted = 0

        def fetch(t):
            """Issue batched loads covering tiles [t, t+2) (dd/ctx/masks) and
            the 2-tile qwin gather."""
            s = {}
            s["ctx"] = io.tile([P, 2 * TE], BF16, tag="ctx", name=f"ctx{t}")
            nc.sync.dma_start(s["ctx"][:], t_ctx[:, t * TE:(t + 2) * TE])
            s["st"] = io.tile([P, 2 * TE], BF16, tag="st", name=f"st{t}")
            nc.sync.dma_start(s["st"][:], t_stm[:, t * TE:(t + 2) * TE])
            s["ss"] = io.tile([P, 2 * TE], BF16, tag="ss", name=f"ss{t}")
            nc.sync.dma_start(s["ss"][:], t_ssm[:, t * TE:(t + 2) * TE])
            s["qw"] = io.tile([P, 2 * P], BF16, tag="qw", name=f"qw{t}")
            # axis=1 -> offset coefficient P: widx holds flat row indices
            # (a_loc * nACB + blk) into qbt viewed as [(P*nACB), P].
            nc.gpsimd.indirect_dma_start(
                out=s["qw"][:], out_offset=None,
                in_=t_qbt[:, :, :],
                in_offset=IndirectOffsetOnAxis(ap=widx[:, t:t + 2], axis=1))
            return s

        rollbuf = {}

        def b_g0(t):
            if t % 8 == 0:
                dd8 = io_dd.tile([2, 8 * TE], BF16, tag="dd", name=f"dd{t}")
                nc.sync.dma_start(dd8[:], t_dd[:, t * TE:(t + 8) * TE])
                sb[("dd", t)] = dd8
            if t % 2 == 0:
                sb[("io", t)] = fetch(t)
            s = {"io": sb[("io", t - t % 2)], "dd": sb[("dd", t - t % 8)],
                 "eighth": t % 8, "half": t % 2}
            sb[t] = s
            h1p = ps_mm.tile([P, TE], F32, tag="mm", name=f"h1p{t}")
            es = slice(s["eighth"] * TE, (s["eighth"] + 1) * TE)
            nc.tensor.matmul(h1p[:], W["wd1aug"], s["dd"][:, es],
                             start=True, stop=True)
            s["h1"] = act.tile([P, TE], BF16, tag="h1", name=f"h1{t}")
            nc.vector.tensor_scalar(s["h1"][:], h1p[:], W["bd1"], 0.0,
                                    op0=ALU.add, op1=ALU.max)
            s["z2"] = ps_mm.tile([P, TE], F32, tag="mm", name=f"z2{t}")
            nc.tensor.matmul(s["z2"][:], W["wd2c"], s["h1"][:],
                             start=True, stop=True)

        def b_g1(t):
            s = sb[t]
            sq2 = act.tile([P, TE], BF16, tag="sq", name=f"sq2{t}")
            nc.scalar.activation(sq2[:], s["z2"][:], AF.Square, bias=zero_b[:])
            vb2 = ps_vb.tile([P, TE], F32, tag="vb", name=f"vb2{t}")
            nc.tensor.matmul(vb2[:], W["umat16"], sq2[:],
                             start=True, stop=True)
            rs2 = act.tile([P, TE], F32, tag="rs", name=f"rs2{t}")
            rsqrt_act(rs2[:], vb2[:], eps_b[:])
            h2 = act.tile([P, TE], BF16, tag="h2", name=f"h2{t}")
            if fastgn:
                nc.vector.scalar_tensor_tensor(
                    out=h2[:], in0=s["z2"][:], scalar=0.0, in1=rs2[:],
                    op0=ALU.max, op1=ALU.mult)
            else:
                tm = act.tile([P, TE], F32, tag="tm2", name=f"tm2{t}")
                nc.vector.tensor_tensor(tm[:], s["z2"][:], rs2[:], op=ALU.mult)
                nc.scalar.activation(h2[:], tm[:], AF.Relu,
                                     scale=gd2w, bias=gd2b)
            s["h2"] = h2

        def b_g2(t):
            s = sb[t]
            hs = slice(s["half"] * TE, (s["half"] + 1) * TE)
            qs = slice(s["half"] * P, (s["half"] + 1) * P)
            c1 = ps_mm.tile([P, TE], F32, tag="mm", name=f"c1{t}")
            nc.tensor.matmul(c1[:], W["w1a"], s["h2"][:],
                             start=True, stop=False)
            nc.tensor.matmul(c1[:], W["w1c"], s["io"]["ctx"][:, hs],
                             start=False, stop=False)
            nc.tensor.matmul(c1[:], s["io"]["qw"][:, qs], s["io"]["st"][:, hs],
                             start=False, stop=True)
            s["c1"] = c1

        def b_g3(t):
            s = sb[t]
            c1 = s["c1"]
            sqc = act.tile([P, TE], BF16, tag="sqc", name=f"sqc{t}")
            nc.scalar.activation(sqc[:], c1[:], AF.Square, bias=zero_b[:])
            if fastgn:
                seg = ps_seg.tile([P, P + 8], F32, tag="seg", name=f"seg{t}")
                s["seg"] = seg
                for k in range(4):
                    nc.tensor.matmul(seg[:, P + k:P + k + 1],
                                     sqc[:, k * P:(k + 1) * P],
                                     W["onescol"], start=True, stop=True)
                rsc = act.tile([P, 4], F32, tag="rsc", name=f"rsc{t}")
                rsqrt_act(rsc[:], seg[:, P:P + 4], eps_b[:])
                s["rsc"] = rsc
                hpc = act.tile([P, TE], BF16, tag="hpc", name=f"hpc{t}")
                nc.vector.tensor_scalar(hpc[:], c1[:], 0.0, None, op0=ALU.max)
                s["hpc"] = hpc
            else:
                vbc = ps_vb.tile([P, TE], F32, tag="vb", name=f"vbc{t}")
                nc.tensor.matmul(vbc[:], W["umat16"], sqc[:],
                                 start=True, stop=True)
                rsf = act.tile([P, TE], F32, tag="rs", name=f"rsf{t}")
                rsqrt_act(rsf[:], vbc[:], eps_b[:])
                tm = act.tile([P, TE], F32, tag="tmc", name=f"tmc{t}")
                nc.vector.tensor_tensor(tm[:], c1[:], rsf[:], op=ALU.mult)
                hpc = act.tile([P, TE], BF16, tag="hpc", name=f"hpc{t}")
                nc.scalar.activation(hpc[:], tm[:], AF.Relu,
                                     scale=gc1w, bias=gc1b)
                s["hpc"] = hpc
                s["rsc"] = None

        def b_g4(t):
            s = sb.pop(t)
            if t % 2 == 1:
                sb.pop(("io", t - 1), None)
            if t % 8 == 7:
                sb.pop(("dd", t - 7), None)
            me = ps_me.tile([P, TE], F32, tag="me", name=f"me{t}")
            for k in range(4):
                nc.tensor.matmul(me[:, k * P:(k + 1) * P],
                                 s["hpc"][:, k * P:(k + 1) * P], W["wc2r"],
                                 start=True, stop=True)
            mes = act.tile([P, TE], BF16, tag="mes", name=f"mes{t}")
            if fastgn:
                for k in range(4):
                    nc.vector.tensor_scalar(
                        mes[:, k * P:(k + 1) * P], me[:, k * P:(k + 1) * P],
                        s["rsc"][:, k:k + 1], None, op0=ALU.mult)
            else:
                nc.vector.tensor_copy(mes[:], me[:])
            if "seg" in s:
                seg = s["seg"]
            else:
                seg = ps_seg.tile([P, P + 8], F32, tag="seg", name=f"seg{t}")
            segp = seg[:, 0:P]
            ss = s["io"]["ss"]
            hb = s["half"] * TE
            for k in range(4):
                nc.tensor.matmul(segp,
                                 ss[:, hb + k * P:hb + (k + 1) * P],
                                 mes[:, k * P:(k + 1) * P],
                                 start=(k == 0), stop=(k == 3))
            r = t - t % 8
            if t % 8 == 0:
                rollbuf[r] = act2.tile([P, 8 * P], BF16, tag="roll",
                                       name=f"roll{r}")
            roll = rollbuf[r]
            nc.vector.tensor_copy(roll[:, (t % 8) * P:(t % 8 + 1) * P],
                                  segp)
            if t % 8 == 7:
                nc.sync.dma_start(t_part[:, r:r + 8, :], roll[:])
                del rollbuf[r]

        if not fastgn:
            phases = [b_g0, b_g1, b_g2, b_g3, b_g4]
            for i in range(nT + len(phases) - 1):
                for d, ph in enumerate(phases):
                    t = i - d
                    if 0 <= t < nT:
                        ph(t)
        else:
            # 12-deep modulo-scheduled pipeline, rebalanced across the four
            # compute engines (ACT/DVE were the old wall at ~87%; Pool was
            # mostly idle).  GPSIMD (Pool) cannot read PSUM on real hw, so
            # z2/c1 get one bf16 evacuation each and Pool does the SBUF-only
            # math (squares + the h2 GN apply).  Bulk loads move to the SP
            # queue in groups of 4 tiles; the qw gather (SWDGE) stays on
            # Pool.
            #   i=t:    h1p mm [PE], h1 relu+bias [DVE]
            #   i=t+1:  z2 mm [PE], z2b copy [ACT]
            #   i=t+2:  sq2 = z2b^2 [Pool]
            #   i=t+3:  vb2 mm [PE], rs2 rsqrt [ACT]
            #   i=t+4:  h2 = relu(z2b)*rs2 [Pool]
            #   i=t+5:  c1 mms [PE], c1b copy [DVE]
            #   i=t+6:  sqc = c1b^2 [Pool], hpc = relu(c1b) [DVE]
            #   i=t+7:  colsum mms [PE], rsc rsqrt [ACT]
            #   i=t+8:  me mms [PE]
            #   i=t+9:  mes evac+scale [DVE k0-1, ACT k2-3]
            #   i=t+10: segp mms [PE]
            #   i=t+11: roll evac [DVE], store/8 [SP]
            # PSUM: ps_mm {z2, c1} x3, ps_vb {h1p, vb2, me} x4,
            # ps_seg pair x1, ps_ssq x2  = 15.1KB/partition.
            def fetch4(a):
                if a >= nT:
                    return
                s = {}
                s["ctx"] = io.tile([P, 4 * TE], BF16, tag="ctx",
                                   name=f"ctx{a}")
                nc.sync.dma_start(s["ctx"][:], t_ctx[:, a * TE:(a + 4) * TE])
                s["st"] = io.tile([P, 4 * TE], BF16, tag="st", name=f"st{a}")
                nc.sync.dma_start(s["st"][:], t_stm[:, a * TE:(a + 4) * TE])
                s["ss"] = io.tile([P, 4 * TE], BF16, tag="ss", name=f"ss{a}")
                nc.sync.dma_start(s["ss"][:], t_ssm[:, a * TE:(a + 4) * TE])
                sb[("io", a)] = s

            def fetch_qw(a):
                if a >= nT:
                    return
                qw = io.tile([P, P], BF16, tag="qw", name=f"qw{a}", bufs=8)
                nc.gpsimd.indirect_dma_start(
                    out=qw[:], out_offset=None,
                    in_=t_qbt[:, :, :],
                    in_offset=IndirectOffsetOnAxis(
                        ap=widx[:, a:a + 1], axis=1))
                sb[("qw", a)] = qw

            def fetch_dd(a):
                if a >= nT:
                    return
                dd8 = io_dd.tile([2, 8 * TE], BF16, tag="dd", name=f"dd{a}")
                nc.sync.dma_start(dd8[:], t_dd[:, a * TE:(a + 8) * TE])
                sb[("dd", a)] = dd8

            fetch_dd(0)
            fetch4(0)
            fetch_qw(0)
            fetch_qw(1)
            fetch_qw(2)

            for i in range(nT + 12):
                # ---------- prefetch (SP-issued) ----------
                if i % 8 == 0:
                    fetch_dd(i + 8)
                if i % 4 == 0:
                    fetch4(i + 4)

                # ---------- PE ----------
                if i < nT:
                    t = i
                    s = {"io": sb[("io", t - t % 4)],
                         "dd": sb[("dd", t - t % 8)]}
                    sb[t] = s
                    h1p = ps_vb.tile([P, TE], F32, tag="vb", name=f"h1p{t}")
                    s["h1p"] = h1p
                    es = slice((t % 8) * TE, (t % 8 + 1) * TE)
                    nc.tensor.matmul(h1p[:], W["wd1aug"], s["dd"][:, es],
                                     start=True, stop=True)
                if 0 <= i - 1 < nT:
                    t = i - 1
                    s = sb[t]
                    z2 = ps_mm.tile([P, TE], F32, tag="mm", name=f"z2{t}")
                    s["z2"] = z2
                    nc.tensor.matmul(z2[:], W["wd2c"], s["h1"][:],
                                     start=True, stop=True)
                if 0 <= i - 3 < nT:
                    t = i - 3
                    s = sb[t]
                    vb2 = ps_vb.tile([P, TE], F32, tag="vb", name=f"vb2{t}")
                    s["vb2"] = vb2
                    nc.tensor.matmul(vb2[:], W["umat16"], s["sq2"][:],
                                     start=True, stop=True)
                if 0 <= i - 5 < nT:
                    t = i - 5
                    s = sb[t]
                    hs = slice((t % 4) * TE, (t % 4 + 1) * TE)
                    c1 = ps_mm.tile([P, TE], F32, tag="mm", name=f"c1{t}")
                    s["c1"] = c1
                    nc.tensor.matmul(c1[:], W["w1c"],
                                     s["io"]["ctx"][:, hs],
                                     start=True, stop=False)
                    nc.tensor.matmul(c1[:], sb[("qw", t)][:],
                                     s["io"]["st"][:, hs],
                                     start=False, stop=False)
                    nc.tensor.matmul(c1[:], W["w1a"], s["h2"][:],
                                     start=False, stop=True)
                if 0 <= i - 7 < nT:
                    t = i - 7
                    s = sb[t]
                    if t % 2 == 0:
                        ssq = ps_ssq.tile([P, 8], F32, tag="ssq",
                                          name=f"ssq{t}")
                        sb[("ssq", t)] = ssq
                    else:
                        ssq = sb[("ssq", t - 1)]
                    sc0 = (t % 2) * 4
                    for k in range(4):
                        nc.tensor.matmul(ssq[:, sc0 + k:sc0 + k + 1],
                                         s["sqc"][:, k * P:(k + 1) * P],
                                         W["onescol"],
                                         start=True, stop=True)
                if 0 <= i - 8 < nT:
                    t = i - 8
                    s = sb[t]
                    me = ps_vb.tile([P, 4, P], F32, tag="vb", name=f"me{t}")
                    s["me"] = me
                    for k in range(4):
                        nc.tensor.matmul(me[:, k, :],
                                         s["hpc"][:, k * P:(k + 1) * P],
                                         W["wc2r"], start=True, stop=True)
                if 0 <= i - 10 < nT:
                    t = i - 10
                    s = sb[t]
                    if t % 2 == 0:
                        segpair = ps_seg.tile([P, 2 * P], F32, tag="seg",
                                              name=f"seg{t}")
                        sb[("segpair", t)] = segpair
                    else:
                        segpair = sb[("segpair", t - 1)]
                    seg = segpair[:, (t % 2) * P:(t % 2 + 1) * P]
                    ss_t = sb[t]["io"]["ss"]
                    hb = (t % 4) * TE
                    for k in range(4):
                        nc.tensor.matmul(seg,
                                         ss_t[:, hb + k * P:hb + (k + 1) * P],
                                         s["mes"][:, k, :],
                                         start=(k == 0), stop=(k == 3))

                # ---------- Pool (SBUF-only math) ----------
                if 0 <= i - 2 < nT:
                    t = i - 2
                    s = sb[t]
                    sq2 = act.tile([P, TE], BF16, tag="sq", name=f"sq2{t}")
                    s["sq2"] = sq2
                    nc.gpsimd.tensor_tensor(sq2[:], s["z2b"][:], s["z2b"][:],
                                            op=ALU.mult)
                if 0 <= i - 4 < nT:
                    t = i - 4
                    s = sb[t]
                    h2 = act.tile([P, TE], BF16, tag="h2", name=f"h2{t}")
                    s["h2"] = h2
                    if t % 2 == 0:
                        u2 = act.tile([P, TE], BF16, tag="u2", name=f"u2{t}")
                        nc.gpsimd.tensor_scalar(u2[:], s["z2b"][:], 0.0,
                                                None, op0=ALU.max)
                        nc.gpsimd.tensor_tensor(h2[:], u2[:], s["rs2"][:],
                                                op=ALU.mult)
                    else:
                        nc.vector.scalar_tensor_tensor(
                            out=h2[:], in0=s["z2b"][:], scalar=0.0,
                            in1=s["rs2"][:], op0=ALU.max, op1=ALU.mult)
                if 0 <= i - 6 < nT:
                    t = i - 6
                    s = sb[t]
                    sqc = act.tile([P, TE], BF16, tag="sqc", name=f"sqc{t}")
                    s["sqc"] = sqc
                    nc.gpsimd.tensor_tensor(sqc[:], s["c1b"][:], s["c1b"][:],
                                            op=ALU.mult)

                # ---------- ACT (ready work first) ----------
                if i < nT:
                    t = i
                    s = sb[t]
                    h1 = act.tile([P, TE], BF16, tag="h1", name=f"h1{t}")
                    s["h1"] = h1
                    nc.scalar.activation(h1[:], s["h1p"][:], AF.Relu,
                                         bias=W["bd1"])
                if 0 <= i - 1 < nT:
                    t = i - 1
                    s = sb[t]
                    z2b = act.tile([P, TE], BF16, tag="z2b", name=f"z2b{t}",
                                   bufs=5)
                    s["z2b"] = z2b
                    nc.scalar.activation(z2b[:], s["z2"][:], AF.Copy)
                if 0 <= i - 3 < nT:
                    t = i - 3
                    s = sb[t]
                    rs2 = act.tile([P, TE], BF16, tag="rs", name=f"rs2{t}")
                    s["rs2"] = rs2
                    rsqrt_act(rs2[:], s["vb2"][:], eps_b[:])
                if 0 <= i - 8 < nT and (i - 8) % 2 == 0:
                    t = i - 8
                    rsc = act4.tile([P, 8], F32, tag="rsc", name=f"rsc{t}")
                    ssq = sb.pop(("ssq", t))
                    sb[("rsc", t)] = rsc
                    rsqrt_act(rsc[:], ssq[:], eps_b[:])

                # ---------- DVE ----------
                if 0 <= i - 11 < nT:
                    t = i - 11
                    r = t - t % 8
                    if t % 8 == 0:
                        rollbuf[r] = act2.tile([P, 8 * P], BF16, tag="roll",
                                               name=f"roll{r}")
                    if t % 2 == 1:
                        segpair = sb.pop(("segpair", t - 1))
                        dst = rollbuf[r][:, (t % 8 - 1) * P:(t % 8 + 1) * P]
                        nc.vector.tensor_copy(dst, segpair[:])
                if 0 <= i - 9 < nT:
                    t = i - 9
                    s = sb[t]
                    mes = act.tile([P, 4, P], BF16, tag="mes", name=f"mes{t}")
                    s["mes"] = mes
                    rsc = sb[("rsc", t - t % 2)]
                    sc0 = (t % 2) * 4
                    rscb = rsc[:, sc0:sc0 + 4, None].broadcast_to([P, 4, P])
                    nc.vector.tensor_tensor(mes[:], s["me"][:], rscb,
                                            op=ALU.mult)
                    if t % 2 == 1:
                        sb.pop(("rsc", t - 1), None)
                if 0 <= i - 6 < nT:
                    t = i - 6
                    s = sb[t]
                    hpc = act4.tile([P, TE], BF16, tag="hpc", name=f"hpc{t}")
                    s["hpc"] = hpc
                    nc.vector.tensor_scalar(hpc[:], s["c1b"][:], 0.0,
                                            None, op0=ALU.max)
                if 0 <= i - 5 < nT:
                    t = i - 5
                    s = sb[t]
                    c1b = act.tile([P, TE], BF16, tag="c1b", name=f"c1b{t}",
                                   bufs=5)
                    s["c1b"] = c1b
                    nc.vector.tensor_copy(c1b[:], s["c1"][:])

                # ---------- qw gather (Pool, last so it can't block) ----
                fetch_qw(i + 3)

                # ---------- SP stores / cleanup ----------
                if 0 <= i - 11 < nT:
                    t = i - 11
                    if t % 8 == 7:
                        r = t - 7
                        nc.sync.dma_start(t_part[:, r:r + 8, :],
                                          rollbuf[r][:])
                        del rollbuf[r]
                    sb.pop(t)
                    sb.pop(("qw", t), None)
                    if t % 4 == 3:
                        sb.pop(("io", t - 3), None)
                    if t % 8 == 7:
                        sb.pop(("dd", t - 7), None)

                # ---------- interleaved stage-C column ----------
                if c_emitted < nAC + 10:
                    ic = c_emitted
                    if ic >= nAC or i >= (cgate[ic] | 7) + 12:
                        emit_c(ic)
                        c_emitted += 1


        for ic in range(c_emitted, nAC + 10):
            emit_c(ic)

    nc.compile()
    return nc


_CACHE = {}


def kernel(agts, ctx, agt_ctrs, ctx_ctrs, hi, wi,
           Wd1, bd1, Wd2, gd2w, gd2b, Wq, gqw, gqb,
           Wc1, gc1w, gc1b, Wc2, Wa, gnw, gnb, Wl, glw, glb,
           _trace=False):
    agts = np.asarray(agts, np.float32)
    ctx = np.asarray(ctx, np.float32)
    agt_ctrs = np.asarray(agt_ctrs, np.float32)
    ctx_ctrs = np.asarray(ctx_ctrs, np.float32)
    hi = np.asarray(hi, np.int32)
    wi = np.asarray(wi, np.int32)

    in_maps, meta = _prepare(agts, ctx, agt_ctrs, ctx_ctrs, hi, wi)
    w = _prep_weights(np.asarray(Wd1, np.float32), np.asarray(bd1, np.float32),
                      np.asarray(Wd2, np.float32), np.asarray(Wq, np.float32),
                      np.asarray(Wc1, np.float32), np.asarray(Wc2, np.float32),
                      np.asarray(Wa, np.float32), np.asarray(Wl, np.float32))
    gvec = np.stack([np.asarray(v, np.float32) for v in
                     [gd2w, gd2b, gqw, gqb, gc1w, gc1b, gnw, gnb, glw, glb]],
                    axis=1)

    fastgn = all(
        np.all(np.asarray(wv, np.float32) == 1.0)
        and np.all(np.asarray(bv, np.float32) == 0.0)
        for wv, bv in [(gd2w, gd2b), (gqw, gqb), (gc1w, gc1b), (gnw, gnb)]
    )
    key = (meta["nT"], meta["nAC"], meta["napad"], fastgn, meta["cgate"])
    if key not in _CACHE:
        _CACHE[key] = _build(key[0], key[1], key[2], fastgn=key[3],
                             cgate=key[4])
    nc = _CACHE[key]

    full_maps = []
    for m in in_maps:
        fm = dict(m)
        fm.update(w)
        fm["gv"] = gvec
        full_maps.append(fm)

    res = run_bass_kernel_spmd(nc, full_maps,
                               core_ids=list(range(NCORES)),
                               trace=_trace)

    out = np.empty((N_AGT, P), np.float32)
    ab = meta["a_bounds"]
    for c in range(NCORES):
        nA = ab[c + 1] - ab[c]
        out[ab[c]:ab[c + 1]] = res.results[c]["out"][:, :nA].T
    if _trace:
        kernel._last_exec_time_ns = res.exec_time_ns
        kernel._last_results = res
    return out

